# revision 1
# baseline (speedup 1.0000x reference)
"""MinGRU Trainium2 kernel (nn_MinGRUTriton_77309411812).

Reference computation (B=4, L=8192, D=1024, fp32):
    gates      = sigmoid(x @ Wg.T + bg)
    candidates = tanh   (x @ Wc.T + bc)
    h_t = gates_t * h_{t-1} + candidates_t        (h_0 = 0, scan along L)

Sharding (8 cores, no cross-core communication):
    core c -> batch b = c // 2, output-channel half eh = c % 2 (512 channels).

Host-side shard prep feeds each core transposed fp16 operands in
DMA-native layouts so the device kernel needs no transposes or casts and
every DMA descriptor is an 8 KB-contiguous per-partition run:
    xh[p, ci, kg, t] = x[b, ci*TC + t, kg*128 + p]   fp16 [128,16,8,512]
    wh[p, kg, e]     = W[eh*512 + e, kg*128 + p]     fp16 [128,8,512]
    (k = kg*128+p is the matmul contraction dim, on partitions)
fp16 operands run the PE at 1 cycle/row (4x faster than fp32) with a
fully-hidden 2-byte LDWEIGHTS and keep absmax relative error ~5e-4
(e5m10 rounding, fp32 PSUM accumulation; |x| < 6, |W| < 0.2, h ~ 5).

The matmul output lands as [e(partitions), t(free)], exactly the layout
tensor_tensor_scan needs (the scan runs along the free dim); h is stored
fp16 in hh[p, ci, eg, t] and un-permuted/upcast on the host.

Per 512-wide t-chunk: one 1 MB DMA load of the x slice (sync queue), 64
accumulating fp16 matmuls (PE), sigmoid/tanh straight out of PSUM with
fused per-partition bias (ACT), one tensor_tensor_scan per 128-channel
group (DVE, chained across chunks via initial=prev[:, -1:]), one 0.5 MB
DMA store.  Weight DMAs ride the scalar-engine HWDGE ring in parallel
with x chunk 0 on the sync ring (the two first-matmul gates); wc and the
x1/x2 prefetches are dep-deferred behind them so their bytes stay out of
the SDMA round-robin.  Dummy matmuls during the DMA wait hold the PE's
HAM clock gate at 2.4 GHz, and the last chunk runs as two 256-wide
halves to shorten the kernel-tail dependency chain.
"""

import sys

import numpy as np

try:
    import concourse.bass as bass  # noqa: F401
except ImportError:  # pragma: no cover - path fallback for fresh environments
    sys.path.insert(0, "/opt/trn_rl_repo")

import concourse.bass as bass
import concourse.mybir as mybir
import concourse.tile as tile
from concourse import bacc
from concourse.bass_utils import run_bass_kernel_spmd
from concourse.tile import add_dep_helper

B, L, D = 4, 8192, 1024
E = D // 2          # output channels per core
N_CORES = 8
TC = 512            # t-chunk (= matmul moving free dim = PSUM bank)
NK = D // 128       # contraction k-groups
NE = E // 128       # output-channel groups per core
NCH = L // TC       # t-chunks

F32 = mybir.dt.float32
F16 = mybir.dt.float16

_compiled = None


def _build():
    nc = bacc.Bacc("TRN2", target_bir_lowering=False, debug=False)

    xh = nc.dram_tensor("xh", [128, NCH, NK, TC], F16, kind="ExternalInput")
    wgh = nc.dram_tensor("wgh", [128, NE, NK, 128], F16, kind="ExternalInput")
    wch = nc.dram_tensor("wch", [128, NE, NK, 128], F16, kind="ExternalInput")
    bias = nc.dram_tensor("bias", [128, 2 * NE], F32, kind="ExternalInput")
    hh = nc.dram_tensor("hh", [128, NCH, NE, TC], F16, kind="ExternalOutput")

    with tile.TileContext(nc) as tc, \
            tc.tile_pool(name="wpool", bufs=1) as wpool, \
            tc.tile_pool(name="xpool", bufs=3) as xpool, \
            tc.tile_pool(name="gcpool", bufs=2) as gcpool, \
            tc.tile_pool(name="hpool", bufs=2) as hpool, \
            tc.tile_pool(name="pspool", bufs=6, space="PSUM") as pspool:

        b_all = wpool.tile([128, 2 * NE], F32)
        nc.sync.dma_start(out=b_all[:], in_=bias[:])
        bg_t = b_all[:, 0:NE]
        bc_t = b_all[:, NE:2 * NE]
        # Startup ordering: the first matmul gates on wg + x chunk 0 only.
        # wg rides the scalar HWDGE ring while x0 rides the sync ring (the
        # two rings drain in parallel); wc/x1/x2 are dep-deferred so their
        # bytes don't compete with the gating transfers.
        # wg arrives as 4 per-e-group pieces: the first matmul unit only
        # gates on piece 0 (256 KB) + x chunk 0, and later pieces stream in
        # behind the running PE.
        wg_t = wpool.tile([128, NE, NK, 128], F16)
        for eg in range(NE):
            i_wg = nc.scalar.dma_start(out=wg_t[:, eg], in_=wgh[:, eg])
        wc_t = wpool.tile([128, NE, NK, 128], F16)
        i_wc = nc.scalar.dma_start(out=wc_t[:], in_=wch[:])
        add_dep_helper(i_wc.ins, i_wg.ins, reason="defer wc behind wg")

        # Warm the PE's HAM clock gate (~3.4us of activity releases the
        # 1.2->2.4 GHz throttle) with dummy matmuls on a zeroed tile while
        # the startup DMAs are in flight.
        warm = wpool.tile([128, 512], F16)
        nc.vector.memset(warm[:], 0.0)
        warm_ps = pspool.tile([128, 512], F32, tag="warm", bufs=1)
        for _ in range(16):
            nc.tensor.matmul(warm_ps[:], warm[:, 0:128], warm[:, 0:512],
                             start=True, stop=True)

        h_prev = None
        for ci in range(NCH):
            x_t = xpool.tile([128, NK, TC], F16, tag="x")
            i_x = nc.sync.dma_start(out=x_t[:], in_=xh[:, ci])
            if ci == 1:
                add_dep_helper(i_x.ins, i_wg.ins, reason="defer x1 behind wg")
            elif ci == 2:
                add_dep_helper(i_x.ins, i_wc.ins, reason="defer x2 behind wc")

            g_t = gcpool.tile([128, NE, TC], F32, tag="g")
            c_t = gcpool.tile([128, NE, TC], F32, tag="c")
            h_t = hpool.tile([128, NE, TC], F16, tag="h")
            last = ci == NCH - 1

            def unit(w_t, b_t, out_t, func, eg, pieces):
                ps = pspool.tile([128, TC], F32, tag="ps", name="ps")
                for toff, tcw in pieces:
                    for kg in range(NK):
                        nc.tensor.matmul(
                            ps[:, toff:toff + tcw],
                            w_t[:, eg, kg, :],
                            x_t[:, kg, toff:toff + tcw],
                            start=(kg == 0),
                            stop=(kg == NK - 1),
                        )
                    nc.scalar.activation(
                        out_t[:, eg, toff:toff + tcw], ps[:, toff:toff + tcw],
                        func, bias=b_t[:, eg:eg + 1],
                    )

            SIG = mybir.ActivationFunctionType.Sigmoid
            TANH = mybir.ActivationFunctionType.Tanh
            whole = ((0, TC),)
            # The very last unit + scan of the kernel run as two 256-wide
            # halves so the final MM->ACT->scan->store chain is half as long.
            halved = ((0, TC // 2), (TC // 2, TC // 2))
            for eg in range(NE):
                unit(wg_t, bg_t, g_t, SIG, eg, whole)
            for eg in range(NE):
                unit(wc_t, bc_t, c_t, TANH, eg,
                     halved if last and eg == NE - 1 else whole)

            for eg in range(NE):
                pieces = halved if last and eg == NE - 1 else whole
                for toff, tcw in pieces:
                    if toff == 0:
                        init = 0.0 if ci == 0 else h_prev[:, eg, TC - 1:TC]
                    else:
                        init = h_t[:, eg, toff - 1:toff]
                    nc.vector.tensor_tensor_scan(
                        h_t[:, eg, toff:toff + tcw],
                        g_t[:, eg, toff:toff + tcw],
                        c_t[:, eg, toff:toff + tcw],
                        initial=init,
                        op0=mybir.AluOpType.mult,
                        op1=mybir.AluOpType.add,
                    )
                    if last:
                        # Per-group stores so the final store (the kernel-
                        # tail gate) only waits on the last scan piece.
                        nc.sync.dma_start(
                            out=hh[:, ci, eg, toff:toff + tcw],
                            in_=h_t[:, eg, toff:toff + tcw],
                        )
            if not last:
                nc.sync.dma_start(out=hh[:, ci], in_=h_t[:])
            h_prev = h_t

    nc.compile()
    return nc


def _get_compiled():
    global _compiled
    if _compiled is None:
        _compiled = _build()
    return _compiled


def make_in_maps(x, Wg, bg, Wc, bc):
    x = np.asarray(x, dtype=np.float32)
    # xh[p, ci, kg, t] = x[b, ci*TC + t, kg*128 + p]
    xhs = [
        np.ascontiguousarray(
            x[b].astype(np.float16)
            .reshape(NCH, TC, NK, 128)
            .transpose(3, 0, 2, 1)
        )
        for b in range(B)
    ]
    in_maps = []
    for c in range(N_CORES):
        b, eh = divmod(c, 2)
        sl = slice(eh * E, (eh + 1) * E)
        # wh[p, eg, kg, e'] = W[eh*512 + eg*128 + e', kg*128 + p]
        wgh = np.ascontiguousarray(
            np.asarray(Wg, np.float32)[sl].astype(np.float16)
            .reshape(NE, 128, NK, 128).transpose(3, 0, 2, 1))
        wch = np.ascontiguousarray(
            np.asarray(Wc, np.float32)[sl].astype(np.float16)
            .reshape(NE, 128, NK, 128).transpose(3, 0, 2, 1))
        in_maps.append({
            "xh": xhs[b],
            "wgh": wgh,
            "wch": wch,
            "bias": np.ascontiguousarray(np.stack(
                [np.asarray(bg, np.float32)[sl].reshape(NE, 128),
                 np.asarray(bc, np.float32)[sl].reshape(NE, 128)],
            ).reshape(2 * NE, 128).T),
        })
    return in_maps


def assemble_output(results):
    out = np.empty((B, L, D), np.float32)
    for c in range(N_CORES):
        b, eh = divmod(c, 2)
        hhv = results[c]["hh"]  # [128, NCH, NE, TC] fp16
        # out[b, ci*TC + t, eh*E + eg*128 + p] = hh[p, ci, eg, t]
        out[b, :, eh * E:(eh + 1) * E] = (
            hhv.transpose(1, 3, 2, 0).reshape(L, E).astype(np.float32))
    return out


def kernel(x, Wg, bg, Wc, bc, _trace=False, _trace_kwargs=None):
    nc = _get_compiled()
    in_maps = make_in_maps(x, Wg, bg, Wc, bc)
    res = run_bass_kernel_spmd(
        nc, in_maps, list(range(N_CORES)), trace=_trace,
        **(_trace_kwargs or {}),
    )
    out = assemble_output(res.results)
    if _trace:
        kernel.last_results = res
    return out



# revision 13
# speedup vs baseline: 1.1098x; 1.1098x over previous
"""MinGRU Trainium2 kernel (nn_MinGRUTriton_77309411812).

Reference computation (B=4, L=8192, D=1024, fp32):
    gates      = sigmoid(x @ Wg.T + bg)
    candidates = tanh   (x @ Wc.T + bc)
    h_t = gates_t * h_{t-1} + candidates_t        (h_0 = 0, scan along L)

Sharding (8 cores, no cross-core communication):
    core c -> batch b = c // 2, output-channel half eh = c % 2 (512 channels).

Precision plan (validated against the fp32 reference by exact host sim):
the candidate path and the gate's k-groups 4-7 run fp16 (1 cycle/row on
the PE); the gate's k-groups 0-3 run fp8-e4m3 in DoubleRow perf mode
(256-wide contraction per matmul, 2 MACs/cell/cycle) which cuts gate PE
time 25%.  Error budget: h error is dominated by the candidate path
(delta-c feeds h directly) while gate noise is suppressed by sigmoid
saturation where |h| is large, so fp8 on half the gate contraction
lands at rel err ~1.7e-2 < 2e-2.  All weights are pre-scaled by 32
(exact power of two) to keep e4m3 values out of the denormal range; the
activation undoes it with scale=1/32 (fp16 partials accumulate in the
same PSUM at the same x32 scale).

Layouts (k = contraction on partitions; host pre-transposes + casts):
    xh [p, ci, kg, t]      fp16 x[b, ci*TC+t, kg*128+p]       [128,16,8,512]
    xp8[p, ci, j, i, t]    e4m3 x[b, ci*TC+t, (2j+i)*128+p]   [128,16,2,2,512]
    wg8[p, eg, j, i, e]    e4m3 32*Wg[E*eh+eg*128+e, (2j+i)*128+p]
    wg16[p, kk, eg, e]     fp16 32*Wg[E*eh+eg*128+e, (4+kk)*128+p]
    wc16[p, kg, eg, e]     fp16 32*Wc[E*eh+eg*128+e, kg*128+p]

The matmul output lands as [e(partitions), t(free)]; sigmoid/tanh run on
ACT straight out of PSUM with fused bias+scale into fp16 g/c (fp16 also
doubles DVE scan throughput); tensor_tensor_scan chains chunks via
initial=prev[:, -1:]; h stores ride the gpsimd DMA ring so the sync ring
carries only x loads (126 GB/s would saturate one ring).

Startup: chunk 0 streams per-piece (xp8 j-pieces then xh kg-pieces on
sync; wg8/wg16/bias/wc16/xp8 prefetches on scalar) and its gate phase
runs kg-outer across 4 PSUM banks so the first matmul starts as soon as
the first 192KB lands instead of waiting for the full 1MB chunk; dummy
matmuls warm the PE's HAM clock (1.2->2.4GHz after ~3.4us of activity)
behind a gpsimd memset.  PE stalls also drop the HAM clock to half for
~3.4us, so chunk prefetches are paced to keep the PE gap-free.  The last
chunk's final unit runs as 256+128+128 pieces to shorten the kernel-tail
MM->ACT->scan->store chain.
"""

import sys

import numpy as np
import ml_dtypes

try:
    import concourse.bass as bass  # noqa: F401
except ImportError:  # pragma: no cover - path fallback for fresh environments
    sys.path.insert(0, "/opt/trn_rl_repo")

import concourse.bass as bass
import concourse.mybir as mybir
import concourse.tile as tile
from concourse import bacc
from concourse.bass_utils import run_bass_kernel_spmd
from concourse.tile import add_dep_helper

B, L, D = 4, 8192, 1024
E = D // 2          # output channels per core
N_CORES = 8
TC = 512            # t-chunk (= matmul moving free dim = PSUM bank)
NK = D // 128       # contraction k-groups
NKF8 = 4            # gate k-groups 0..NKF8-1 run fp8 DoubleRow
NJ = NKF8 // 2      # DoubleRow units (256-wide contraction each)
KK = NK - NKF8      # gate fp16 k-groups
NE = E // 128       # output-channel groups per core
NCH = L // TC       # t-chunks
WSCALE = 32.0       # weight pre-scale (exact power of two)

F32 = mybir.dt.float32
F16 = mybir.dt.float16
F8 = mybir.dt.float8e4
DR = mybir.MatmulPerfMode.DoubleRow

_compiled = None


def _build():
    nc = bacc.Bacc("TRN2", target_bir_lowering=False, debug=False)

    xh = nc.dram_tensor("xh", [128, NCH, NK, TC], F16, kind="ExternalInput")
    xp8 = nc.dram_tensor("xp8", [128, NCH, NJ, 2, TC], F8, kind="ExternalInput")
    wg8 = nc.dram_tensor("wg8", [128, NE, NJ, 2, 128], F8, kind="ExternalInput")
    wg16 = nc.dram_tensor("wg16", [128, KK, NE, 128], F16, kind="ExternalInput")
    wc16 = nc.dram_tensor("wc16", [128, NK, NE, 128], F16, kind="ExternalInput")
    bias = nc.dram_tensor("bias", [128, 2 * NE], F32, kind="ExternalInput")
    hh = nc.dram_tensor("hh", [128, NCH, NE, TC], F16, kind="ExternalOutput")

    with tile.TileContext(nc) as tc, \
            tc.tile_pool(name="wpool", bufs=1) as wpool, \
            tc.tile_pool(name="xpool", bufs=3) as xpool, \
            tc.tile_pool(name="gcpool", bufs=2) as gcpool, \
            tc.tile_pool(name="hpool", bufs=2) as hpool, \
            tc.tile_pool(name="pspool", bufs=6, space="PSUM") as pspool:

        # ---- startup DMAs, need-ordered ----------------------------------
        # scalar ring: wg8 -> wg16 -> bias -> wc16 (per-kg; chunk-0 cand is
        #   kg-outer so kg pieces are consumed in arrival order).
        # sync ring:   chunk 0-2 streamed per piece in first-use order
        #   (xp8 j-pieces, xh gate k-groups 4-7, then 0-3); later x loads
        #   are dep-deferred so their bytes don't steal SDMA bandwidth from
        #   the weight transfers that gate earlier compute.
        wg8_t = wpool.tile([128, NE, NJ, 2, 128], F8)
        for eg in range(NE):
            i_wg8 = nc.scalar.dma_start(out=wg8_t[:, eg], in_=wg8[:, eg])
        wg16_t = wpool.tile([128, KK, NE, 128], F16)
        for kk in range(KK):
            i_wg16 = nc.scalar.dma_start(out=wg16_t[:, kk], in_=wg16[:, kk])
        b_all = wpool.tile([128, 2 * NE], F32)
        nc.scalar.dma_start(out=b_all[:], in_=bias[:])
        bg_t = b_all[:, 0:NE]
        bc_t = b_all[:, NE:2 * NE]
        CAND_KG = list(range(NKF8, NK)) + list(range(NKF8))
        wc16_t = wpool.tile([128, NK, NE, 128], F16)
        for kg in CAND_KG:
            i_wc = nc.scalar.dma_start(out=wc16_t[:, kg], in_=wc16[:, kg])

        # Warm the PE's HAM clock gate with dummy matmuls behind a gpsimd
        # memset; just enough to cover until the first gate operands land.
        warm = wpool.tile([128, 512], F16)
        nc.gpsimd.memset(warm[:], 0.0)
        warm_ps = pspool.tile([128, 512], F32, tag="warm", bufs=1)
        for _ in range(2):
            nc.tensor.matmul(warm_ps[:], warm[:, 0:128], warm[:, 0:512],
                             start=True, stop=True)

        SIG = mybir.ActivationFunctionType.Sigmoid
        TANH = mybir.ActivationFunctionType.Tanh
        INV = 1.0 / WSCALE

        h_prev = None
        for ci in range(NCH):
            xp8_t = xpool.tile([128, NJ, 2, TC], F8, tag="xp8")
            x_t = xpool.tile([128, NK, TC], F16, tag="x")
            if ci < 3:
                # per-piece streaming in first-use order
                for j in range(NJ):
                    i_p = nc.sync.dma_start(out=xp8_t[:, j], in_=xp8[:, ci, j])
                    if ci == 1 and j == 0:
                        # keep sync quiet while wc16 streams for chunk-0 cand
                        add_dep_helper(i_p.ins, i_wc.ins,
                                       reason="defer x1 behind wc")
                for kg in list(range(NKF8, NK)) + list(range(NKF8)):
                    i_xkg = nc.sync.dma_start(out=x_t[:, kg], in_=xh[:, ci, kg])
                    if ci == 0 and kg == 0:
                        # cand k-groups yield sync bandwidth to the gate
                        # weights until those are in
                        add_dep_helper(i_xkg.ins, i_wg16.ins,
                                       reason="defer x0 cand-kgs behind wg16")
            else:
                nc.sync.dma_start(out=xp8_t[:], in_=xp8[:, ci])
                nc.sync.dma_start(out=x_t[:], in_=xh[:, ci])

            g_t = gcpool.tile([128, NE, TC], F16, tag="g")
            c_t = gcpool.tile([128, NE, TC], F16, tag="c")
            h_t = hpool.tile([128, NE, TC], F16, tag="h")
            last = ci == NCH - 1

            def gate_mms(ps, eg, toff, tcw):
                # one gate accumulation: 2 fp8 DoubleRow + KK fp16 matmuls
                for j in range(NJ):
                    nc.tensor.matmul(
                        ps[:, toff:toff + tcw],
                        wg8_t[:, eg, j],
                        xp8_t[:, j, :, toff:toff + tcw],
                        start=(j == 0), stop=False,
                        perf_mode=DR,
                    )
                for kk in range(KK):
                    nc.tensor.matmul(
                        ps[:, toff:toff + tcw],
                        wg16_t[:, kk, eg],
                        x_t[:, NKF8 + kk, toff:toff + tcw],
                        start=False, stop=(kk == KK - 1),
                    )

            def cand_mms(ps, eg, toff, tcw):
                for kg in range(NK):
                    nc.tensor.matmul(
                        ps[:, toff:toff + tcw],
                        wc16_t[:, kg, eg],
                        x_t[:, kg, toff:toff + tcw],
                        start=(kg == 0), stop=(kg == NK - 1),
                    )

            if ci == 0:
                # kg-outer across 4 PSUM banks: each arriving piece feeds 4
                # back-to-back matmuls, so the PE streams with the DMA.
                ps_g = [pspool.tile([128, TC], F32, tag="ps", name="ps")
                        for _ in range(NE)]
                for j in range(NJ):
                    for eg in range(NE):
                        nc.tensor.matmul(
                            ps_g[eg][:], wg8_t[:, eg, j], xp8_t[:, j],
                            start=(j == 0), stop=False, perf_mode=DR,
                        )
                for kk in range(KK):
                    for eg in range(NE):
                        nc.tensor.matmul(
                            ps_g[eg][:], wg16_t[:, kk, eg], x_t[:, NKF8 + kk],
                            start=False, stop=(kk == KK - 1),
                        )
                for eg in range(NE):
                    nc.scalar.activation(
                        g_t[:, eg], ps_g[eg][:], SIG,
                        bias=bg_t[:, eg:eg + 1], scale=INV,
                    )
                ps_c = [pspool.tile([128, TC], F32, tag="ps", name="ps")
                        for _ in range(NE)]
                for n, kg in enumerate(CAND_KG):
                    for eg in range(NE):
                        nc.tensor.matmul(
                            ps_c[eg][:], wc16_t[:, kg, eg], x_t[:, kg],
                            start=(n == 0), stop=(n == NK - 1),
                        )
                for eg in range(NE):
                    nc.scalar.activation(
                        c_t[:, eg], ps_c[eg][:], TANH,
                        bias=bc_t[:, eg:eg + 1], scale=INV,
                    )
            else:
                whole = ((0, TC),)
                # last unit of the kernel runs in shrinking pieces so the
                # final MM->ACT->scan->store chain is short
                tail_pieces = ((0, 256), (256, 128), (384, 128))
                for eg in range(NE):
                    ps = pspool.tile([128, TC], F32, tag="ps", name="ps")
                    gate_mms(ps, eg, 0, TC)
                    nc.scalar.activation(
                        g_t[:, eg], ps[:], SIG,
                        bias=bg_t[:, eg:eg + 1], scale=INV,
                    )
                for eg in range(NE):
                    pieces = tail_pieces if last and eg == NE - 1 else whole
                    ps = pspool.tile([128, TC], F32, tag="ps", name="ps")
                    for toff, tcw in pieces:
                        cand_mms(ps, eg, toff, tcw)
                        nc.scalar.activation(
                            c_t[:, eg, toff:toff + tcw],
                            ps[:, toff:toff + tcw], TANH,
                            bias=bc_t[:, eg:eg + 1], scale=INV,
                        )

            for eg in range(NE):
                pieces = (tail_pieces if (last and eg == NE - 1 and ci > 0)
                          else ((0, TC),))
                for toff, tcw in pieces:
                    if toff == 0:
                        init = 0.0 if ci == 0 else h_prev[:, eg, TC - 1:TC]
                    else:
                        init = h_t[:, eg, toff - 1:toff]
                    nc.vector.tensor_tensor_scan(
                        h_t[:, eg, toff:toff + tcw],
                        g_t[:, eg, toff:toff + tcw],
                        c_t[:, eg, toff:toff + tcw],
                        initial=init,
                        op0=mybir.AluOpType.mult,
                        op1=mybir.AluOpType.add,
                    )
                    if last:
                        # per-piece stores: the final store (kernel-tail
                        # gate) only waits on the last scan piece
                        nc.gpsimd.dma_start(
                            out=hh[:, ci, eg, toff:toff + tcw],
                            in_=h_t[:, eg, toff:toff + tcw],
                        )
            if not last:
                nc.gpsimd.dma_start(out=hh[:, ci], in_=h_t[:])
            h_prev = h_t

    nc.compile()
    return nc


def _get_compiled():
    global _compiled
    if _compiled is None:
        _compiled = _build()
    return _compiled


def make_in_maps(x, Wg, bg, Wc, bc):
    x = np.asarray(x, dtype=np.float32)
    E4 = ml_dtypes.float8_e4m3
    # xh[p, ci, kg, t] = x[b, ci*TC + t, kg*128 + p]
    xhs = [
        np.ascontiguousarray(
            x[b].astype(np.float16)
            .reshape(NCH, TC, NK, 128)
            .transpose(3, 0, 2, 1)
        )
        for b in range(B)
    ]
    # xp8[p, ci, j, i, t] = e4m3(x[b, ci*TC + t, (2j+i)*128 + p])
    xp8s = [
        np.ascontiguousarray(
            x[b, :, :NKF8 * 128].astype(E4)
            .reshape(NCH, TC, NJ, 2, 128)
            .transpose(4, 0, 2, 3, 1)
        )
        for b in range(B)
    ]
    in_maps = []
    for c in range(N_CORES):
        b, eh = divmod(c, 2)
        sl = slice(eh * E, (eh + 1) * E)
        wgs = np.asarray(Wg, np.float32)[sl] * WSCALE
        wcs = np.asarray(Wc, np.float32)[sl] * WSCALE
        # wg8[p, eg, j, i, e] = e4m3(32*Wg[., (2j+i)*128+p]) for k-groups 0..3
        wg8 = np.ascontiguousarray(
            wgs[:, :NKF8 * 128].astype(E4)
            .reshape(NE, 128, NJ, 2, 128).transpose(4, 0, 2, 3, 1))
        # wg16[p, kk, eg, e] = fp16(32*Wg[., (NKF8+kk)*128+p])
        wg16 = np.ascontiguousarray(
            wgs[:, NKF8 * 128:].astype(np.float16)
            .reshape(NE, 128, KK, 128).transpose(3, 2, 0, 1))
        # wc16[p, kg, eg, e] = fp16(32*Wc[., kg*128+p])
        wc16 = np.ascontiguousarray(
            wcs.astype(np.float16)
            .reshape(NE, 128, NK, 128).transpose(3, 2, 0, 1))
        in_maps.append({
            "xh": xhs[b],
            "xp8": xp8s[b],
            "wg8": wg8,
            "wg16": wg16,
            "wc16": wc16,
            "bias": np.ascontiguousarray(np.stack(
                [np.asarray(bg, np.float32)[sl].reshape(NE, 128),
                 np.asarray(bc, np.float32)[sl].reshape(NE, 128)],
            ).reshape(2 * NE, 128).T),
        })
    return in_maps


def assemble_output(results):
    out = np.empty((B, L, D), np.float32)
    for c in range(N_CORES):
        b, eh = divmod(c, 2)
        hhv = results[c]["hh"]  # [128, NCH, NE, TC] fp16
        # out[b, ci*TC + t, eh*E + eg*128 + p] = hh[p, ci, eg, t]
        out[b, :, eh * E:(eh + 1) * E] = (
            hhv.transpose(1, 3, 2, 0).reshape(L, E).astype(np.float32))
    return out


def kernel(x, Wg, bg, Wc, bc, _trace=False, _trace_kwargs=None):
    nc = _get_compiled()
    in_maps = make_in_maps(x, Wg, bg, Wc, bc)
    res = run_bass_kernel_spmd(
        nc, in_maps, list(range(N_CORES)), trace=_trace,
        **(_trace_kwargs or {}),
    )
    out = assemble_output(res.results)
    if _trace:
        kernel.last_results = res
    return out
